# revision 32
# baseline (speedup 1.0000x reference)
"""KindredAttention on 8 trn2 NeuronCores.

Sharding: core(b, g) = b*2 + g for batch b in 0..3, head-group g in 0..1
(8 heads per group). Data-parallel over batch, tensor-parallel over heads
(qkv column-split, o_proj row-split; host sums the two o_proj partials).

Per-core layouts (host pre-transposes so the kernel never transposes):
  xt   [1024, 2048] bf16 : hidden[b].T                  (d-major)
  wqkv [1024, 1536] bf16 : qkv_w rows for this group, transposed.
                           cols = [q(8 heads x 64) | k(...) | v(...)]
  owt  [512, 1024]  bf16 : o_w[:, group cols].T
  cos/sina [128, 2048] bf16 : RoPE tables (2 heads stacked, sign-folded sin)
  out  [2048, 1024] f32  : partial o_proj output (host adds g=0 + g=1)

v2 changes vs baseline:
  - Scores run head-PAIRS concurrently via PE row tiling (contraction=64:
    head 2f on array rows 0-63, head 2f+1 on rows 64-127, tile_position
    auto-derived from base partitions) -> scores at full PE rate.
  - One static PSUM layout for the whole kernel (pool A: 4x[128,512],
    pool B: 2x[128,1024] = 8 banks); phase-1 qkv, attention scores,
    attention PV and o_proj all share it, so no pool-close barriers.
  - wq / xt DMAs split per-dc (contiguous 384KB / 256KB chunks) so the
    first projection matmul issues after ~1 chunk instead of the full 5MB.
  - exp split per (chunk, head, j-half) across ACT(36)/DVE(28) per pair;
    softmax recip runs directly on PSUM row 64 (vector.reciprocal),
    denominator broadcast on GpSimd, normalize muls deferred into the
    next pair's stream so no engine FIFO stalls the PE.
  - phase-1 sb1 fc order interleaved q/k (0,4,1,5,...) so attention pair 0
    unblocks as soon as its own q/k rope is done.
"""

import os

import ml_dtypes
import numpy as np

H = 16
D = 64
BASE = 10000.0
B, S, HD = 4, 2048, 1024
G = 2          # head groups (tensor parallel)
HG = H // G    # heads per group = 8
N_CORES = 8

last_results = None  # stash for test.py (exec_time_ns etc.)


def _rope_tables():
    inv_freq = 1.0 / (BASE ** (np.arange(0, D, 2, dtype=np.float32) / D))
    t = np.arange(S, dtype=np.float32)
    freqs = np.outer(t, inv_freq)                       # [S, 32]
    emb = np.concatenate([freqs, freqs], -1)            # [S, 64]
    cos = np.cos(emb).T.astype(np.float32)              # [64, S]
    sin = np.sin(emb).T.astype(np.float32)
    sina = sin.copy()
    sina[:32] = -sina[:32]                              # rotate_half sign fold
    # swap 32-row halves: row p holds the factor applied to source row p, so
    # the DVE rot-mul reads src and table at the SAME base partition
    # (SBUF-SBUF TensorTensor requires equal input base partitions).
    sins = np.concatenate([sina[32:], sina[:32]], axis=0)
    cos128 = np.tile(cos, (2, 1)).astype(ml_dtypes.bfloat16).copy()
    sins128 = np.tile(sins, (2, 1)).astype(ml_dtypes.bfloat16).copy()
    return cos128, sins128


def _build():
    import concourse.mybir as mybir
    import concourse.tile as tile
    from concourse import bacc

    F32 = mybir.dt.float32
    BF16 = mybir.dt.bfloat16
    I16 = mybir.dt.int16
    Exp = mybir.ActivationFunctionType.Exp
    Copy = mybir.ActivationFunctionType.Copy
    Mult = mybir.AluOpType.mult
    Add = mybir.AluOpType.add

    # Schraudolph exp in bf16 bit space (scale 1/8 folded):
    #   bits16 = round(s * 16/ln2 + C)  ->  bitcast bf16 ~= exp(s/8)
    EXPA = 0.125 * 128.0 / float(np.log(2.0))
    EXPC = 16248.60

    nc = bacc.Bacc("TRN2", target_bir_lowering=False, debug=False,
                   num_devices=N_CORES)
    xt_d = nc.dram_tensor("xt", [HD, S], BF16, kind="ExternalInput")
    wq_d = nc.dram_tensor("wqkv", [HD, 3 * HG * D], BF16, kind="ExternalInput")
    ow_d = nc.dram_tensor("owt", [HG * D, HD], BF16, kind="ExternalInput")
    cos_d = nc.dram_tensor("cos", [128, S], BF16, kind="ExternalInput")
    sina_d = nc.dram_tensor("sina", [128, S], BF16, kind="ExternalInput")
    out_d = nc.dram_tensor("out", [S, HD], F32, kind="ExternalOutput")

    QB = S // 1024  # 2 big q/s blocks
    SC = S // 128   # 16 k-chunks
    DC = HD // 128  # 8 d-chunks

    with tile.TileContext(nc) as tc:
        with (
            tc.tile_pool(name="persist", bufs=1) as persist,
            tc.tile_pool(name="w1", bufs=1) as w1p,
            tc.tile_pool(name="xts", bufs=2) as xtp,
            tc.tile_pool(name="rope", bufs=5) as ropep,
            # PSUM: pool A = 4 x [128,512] (qkv proj j-halves, scores,
            # o_proj), pool B = 2 x [128,1024] (v proj, PV accum). 8 banks.
            tc.tile_pool(name="psA", bufs=1, space="PSUM") as psA,
            tc.tile_pool(name="psB", bufs=1, space="PSUM") as psB,
            tc.tile_pool(name="es", bufs=4) as esp,
            tc.tile_pool(name="nrm", bufs=1) as nrmp,
            tc.tile_pool(name="ot", bufs=2) as otp,
            tc.tile_pool(name="og", bufs=2) as ogp,
        ):
            # PE warm-up: ~20 throwaway matmuls on a zeroed tile while the
            # first DMAs land, so the HAM clock gate releases (K=8/8)
            # before real work starts instead of ~7us into it.
            wu_sb = ropep.tile([128, 512], BF16, tag="t00", name="wu_sb")
            nc.gpsimd.memset(wu_sb[:], 0.0)
            wu_ps = psA.tile([128, 2, 512], F32, tag="A0", name="wu_ps")
            for _ in range(20):
                nc.tensor.matmul(wu_ps[:, 0, :], wu_sb[:, 0:128], wu_sb[:],
                                 start=True, stop=True)

            # q rows: fc 0-3, k rows: fc 4-7 (feature-major, 2 heads/tile)
            qk_sb = persist.tile([128, 8, S], BF16, tag="qk")
            v_sb = persist.tile([128, SC, HG, D + 1], BF16, tag="v")
            nc.vector.memset(v_sb[:], 1.0)  # ones column survives at [..., 64]

            # RoPE tables land in compact [128,512] tiles per (sb, j):
            # reading 512-col slices of a 2048-wide tile runs the DVE
            # TensorTensor at ~1/3 rate (strided src), compact tiles don't.
            cos_t, sina_t = [], []
            for sj in range(4):
                c4 = w1p.tile([128, 512], BF16, tag=f"cos{sj}",
                              name=f"cos{sj}")
                s4_ = w1p.tile([128, 512], BF16, tag=f"sina{sj}",
                               name=f"sina{sj}")
                cos_t.append(c4)
                sina_t.append(s4_)
            ow_sb = w1p.tile([128, 4, HD], BF16, tag="ow")

            # ------------- DMA: per-dc chunks so compute starts early ------
            def load_xts(sb, wq_out=None):
                ssl = slice(sb * 1024, (sb + 1) * 1024)
                ts = []
                for dc in range(DC):
                    if wq_out is not None:  # interleave wq/xt per dc
                        wt = w1p.tile([128, 3 * HG * D], BF16,
                                      tag=f"wq{dc}", name=f"wq{dc}")
                        nc.sync.dma_start(
                            wt[:], wq_d[dc * 128:(dc + 1) * 128, :])
                        wq_out.append(wt)
                    xt_t = xtp.tile([128, 1024], BF16, tag=f"xt{dc}")
                    nc.sync.dma_start(
                        xt_t[:], xt_d[dc * 128:(dc + 1) * 128, ssl])
                    ts.append(xt_t)
                return ts

            wq_t = []
            xts0 = load_xts(0, wq_out=wq_t)
            for sj in range(4):
                ssl = slice(sj * 512, (sj + 1) * 512)
                nc.sync.dma_start(cos_t[sj][:], cos_d[:, ssl])
                nc.sync.dma_start(sina_t[sj][:], sina_d[:, ssl])
            nc.sync.dma_start(
                ow_sb[:], ow_d[:].rearrange("(a p) f -> p a f", p=128))



            # ---------------- phase 1: qkv projection + RoPE ----------------
            def fc_loop(sb, xts, fc_order):
                for fi, fc in enumerate(fc_order):
                    ps = psA.tile([128, 2, 512], F32, tag=f"A{fi % 2}",
                                  name="psqk")
                    for j in range(2):
                        for dc in range(DC):
                            nc.tensor.matmul(
                                ps[:, j, :],
                                wq_t[dc][:, fc * 128:(fc + 1) * 128],
                                xts[dc][:, j * 512:(j + 1) * 512],
                                start=(dc == 0), stop=(dc == DC - 1),
                            )
                    for j in range(2):
                        sj = sb * 2 + j
                        jsl = slice(sb * 1024 + j * 512,
                                    sb * 1024 + (j + 1) * 512)
                        # drain to bf16 on ACT (idle in phase 1)
                        t0 = ropep.tile([128, 512], BF16, tag=f"t0{j}")
                        nc.scalar.activation(t0[:], ps[:, j, :], Copy)
                        t1 = ropep.tile([128, 512], BF16, tag=f"t1{j}")
                        for r in (0, 64):  # two heads per chunk
                            nc.vector.tensor_mul(
                                t1[r:r + 32, :], t0[r + 32:r + 64, :],
                                sina_t[sj][r + 32:r + 64, :])
                            nc.vector.tensor_mul(
                                t1[r + 32:r + 64, :], t0[r:r + 32, :],
                                sina_t[sj][r:r + 32, :])
                        m1 = ropep.tile([128, 512], BF16, tag=f"m1{j}")
                        nc.vector.tensor_mul(m1[:], t0[:], cos_t[sj][:])
                        # final add on GpSimd: trims the DVE rope backlog so
                        # it drains before the attention exps need the DVE
                        nc.gpsimd.tensor_add(qk_sb[:, fc, jsl], m1[:], t1[:])

            def v_loop(sb, xts):
                for s4 in range(8):  # v s-chunks in this block
                    sc = sb * 8 + s4
                    ps = psB.tile([128, 1024], F32, tag=f"B{s4 % 2}")
                    for dc in range(DC):
                        nc.tensor.matmul(
                            ps[:, 0:512],
                            xts[dc][:, s4 * 128:(s4 + 1) * 128],
                            wq_t[dc][:, 1024:1536],
                            start=(dc == 0), stop=(dc == DC - 1),
                        )
                    nc.scalar.activation(
                        v_sb[:, sc, :, 0:D],
                        ps[:, 0:512].rearrange("p (h d) -> p h d", d=D),
                        Copy)

            xts1 = load_xts(1)
            fc_loop(0, xts0, [0, 4, 1, 5, 2, 6, 3, 7])
            v_loop(0, xts0)
            fc_loop(1, xts1, [0, 4, 1, 5, 2, 6, 3, 7])
            v_loop(1, xts1)  # 17us of DVE-free PE: drains the rope backlog

            # ---------------- phase 2+3: attention + o_proj ----------------
            # Per (q-block 512, head-pair f = heads 2f,2f+1):
            #   scores: K-chunk stationary [64,128] on array rows 0-63 (even
            #   head) / 64-127 (odd head) -> the two MMs run CONCURRENT via
            #   row tiling, into one [128, 2, 512] PSUM tile (pair side by
            #   side). ONE [128,1024] exp op per chunk, strictly alternating
            #   ACT (even c) / DVE (odd c) so the engines overlap, with a
            #   2-chunk PSUM pipeline. PV accumulates the ones-augmented V
            #   for both heads into one [128,1024] tile (head p at cols
            #   p*512) so the denominator row, broadcast and normalize run
            #   once per pair; normalize is an exact DVE divide (no recip).
            pending_norm = []  # deferred (ci_target, fn) normalize stages

            def flush_norm(ci=None):
                while pending_norm and (ci is None or pending_norm[0][0] <= ci):
                    pending_norm.pop(0)[1]()

            def att_unit(qb, qk2, f, ot, act_only=False):
                qsl = slice(qb * 1024 + qk2 * 512,
                            qb * 1024 + (qk2 + 1) * 512)
                osl = slice(qk2 * 512, (qk2 + 1) * 512)
                pv = psB.tile([128, 1024], F32, tag=f"B{f % 2}",
                              name="pv")
                es_t = [None] * SC
                # chunks run in GROUPS of 2: scores(c), scores(c+1) back to
                # back (their LDWs background-load within the same row
                # groups), then pv(c-2), pv(c-1) -> half the scores<->PV
                # LDW row-conflict serializations per chunk.
                for gi in range(SC // 2 + 1):
                    for c in (2 * gi, 2 * gi + 1):
                        if c >= SC:
                            continue
                        csl = slice(c * 128, (c + 1) * 128)
                        # score PSUM rotation: depth 2 (A0/A1) for early
                        # chunks; from c>=8 the idle B bank (prev unit's
                        # pv, freed by its staged muls) joins -> depth 3,
                        # so exp(c) gates scores of c+3 instead of c+2.
                        # c=14/15 land on A0/A1 so the NEXT unit's first
                        # chunks don't wait on this unit's last exps.
                        tag3 = (f"A{c % 2}" if c < 8 else
                                ['A0', 'A1', 'B!'][(c - 8) % 3])
                        if tag3 == 'B!':
                            q_ps = psB.tile(
                                [128, 1024], F32,
                                tag=f"B{(f + 1) % 2}", name="qsb")
                            outs = [q_ps[:, 0:512], q_ps[:, 512:1024]]
                            flat = q_ps[:]
                        else:
                            q_ps = psA.tile([128, 2, 512], F32,
                                            tag=tag3, name="qs")
                            outs = [q_ps[:, 0, :], q_ps[:, 1, :]]
                            flat = q_ps[:].rearrange("p a b -> p (a b)")
                        for p in range(2):  # head parity: rows p*64
                            r = slice(p * 64, (p + 1) * 64)
                            nc.tensor.matmul(
                                outs[p],
                                qk_sb[r, 4 + f, csl],
                                qk_sb[r, f, qsl],
                                start=True, stop=True,
                            )
                        es2 = esp.tile([128, 2, 512], BF16,
                                       tag="es", name="es")
                        if c % 2 == 0 or act_only:  # ACT even / DVE odd
                            nc.scalar.activation(
                                es2[:].rearrange("p a b -> p (a b)"),
                                flat, Exp, scale=0.125)
                        else:
                            with nc.allow_low_precision(
                                    reason="schraudolph exp bf16"):
                                nc.vector.tensor_scalar(
                                    es2[:].rearrange(
                                        "p a b -> p (a b)"
                                    ).bitcast(I16),
                                    flat, EXPA, EXPC, Mult, Add)
                        es_t[c] = es2
                        flush_norm(c)  # prev unit's staged chain
                    for c in (2 * gi - 2, 2 * gi - 1):
                        if c < 0:
                            continue
                        for p in range(2):
                            nc.tensor.matmul(
                                pv[0:D + 1, p * 512:(p + 1) * 512],
                                v_sb[:, c, 2 * f + p, :],
                                es_t[c][:, p, :],
                                start=(c == 0), stop=(c == SC - 1),
                            )
                # softmax denominators (both heads at once): PSUM row
                # 64 -> SBUF on ACT, partition-broadcast on GpSimd,
                # reciprocal on the BROADCAST (full 64 partitions, much
                # cheaper than a 1-partition recip) on DVE, then two DVE
                # muls; the chain is staged into the next unit's stream
                # so no engine FIFO blocks.
                den = nrmp.tile([1, 1024], F32, tag="den")
                nc.scalar.activation(den[:], pv[D:D + 1, :], Copy)

                bc_box = []
                rc_box = []

                def st1(den_t=den, bc_box=bc_box):
                    bcs = nrmp.tile([64, 1024], F32, tag="bcs",
                                    name="bcs")
                    nc.gpsimd.partition_broadcast(bcs[:], den_t[:])
                    bc_box.append(bcs)

                def st2(bc_box=bc_box, rc_box=rc_box):
                    rcb = nrmp.tile([64, 1024], F32, tag="rcb",
                                    name="rcb")
                    with nc.allow_low_precision(reason="softmax recip"):
                        nc.vector.reciprocal_approx_fast(
                            rcb[:], bc_box[0][:])
                    rc_box.append(rcb)

                def st3(f=f, pv_t=pv, rc_box=rc_box, ot_t=ot, osl=osl):
                    for p in range(2):
                        ps512 = slice(p * 512, (p + 1) * 512)
                        nc.vector.tensor_mul(
                            ot_t[p * 64:(p + 1) * 64, f, osl],
                            pv_t[0:D, ps512], rc_box[0][:, ps512])
                pending_norm.extend([(1, st1), (4, st2), (6, st3)])

            def o_proj(qb, ot):
                # po shares pool-A banks with the unit scores
                for scq in range(8):
                    po = psA.tile([128, 2, 512], F32, tag=f"A{scq % 2}",
                                  name="po")
                    for jf in range(2):
                        jsl = slice(jf * 512, (jf + 1) * 512)
                        for oc in range(4):
                            nc.tensor.matmul(
                                po[:, jf, :],
                                ot[:, oc, scq * 128:(scq + 1) * 128],
                                ow_sb[:, oc, jsl],
                                start=(oc == 0), stop=(oc == 3),
                            )
                    og = ogp.tile([128, 1024], F32, tag="og")
                    nc.scalar.activation(
                        og[:], po[:].rearrange("p a b -> p (a b)"), Copy)
                    nc.sync.dma_start(
                        out_d[qb * 1024 + scq * 128:
                              qb * 1024 + (scq + 1) * 128, :], og[:])

            # Driver: qb0's o_proj is emitted AFTER qb1's first unit, so the
            # last qb0 unit's normalize chain (den->bcast->recip->muls,
            # ~6us of cross-engine latency) hides under that unit's PE work
            # instead of stalling the PE.  qb1's tail pays it once.
            ots = {0: otp.tile([128, 4, 1024], BF16, tag="ot", name="ot0"),
                   1: otp.tile([128, 4, 1024], BF16, tag="ot", name="ot1")}
            units = [(qb, qk2, f) for qb in range(QB) for qk2 in range(2)
                     for f in range(4)]
            for i, (qb, qk2, f) in enumerate(units):
                # the very first unit overlaps sb1's DVE rope tail: keep its
                # exps off the DVE so nothing stalls
                att_unit(qb, qk2, f, ots[qb],
                         act_only=(i == 0))
                if i == 9:
                    o_proj(0, ots[0])
            flush_norm()
            o_proj(1, ots[1])

    nc.compile()
    return nc


def kernel(hidden_states, qkv_w, o_w):
    global last_results
    from concourse.bass_utils import run_bass_kernel_spmd

    hidden_states = np.asarray(hidden_states, dtype=np.float32)
    qkv_w = np.asarray(qkv_w, dtype=np.float32)
    o_w = np.asarray(o_w, dtype=np.float32)

    cos128, sina128 = _rope_tables()
    nc = _build()

    in_maps = []
    for core in range(N_CORES):
        b, g = core // G, core % G
        heads = range(g * HG, (g + 1) * HG)
        rows = np.concatenate(
              [np.arange(h * D, (h + 1) * D) for h in heads])
        wsel = np.concatenate(
              [qkv_w[off + rows] for off in (0, HD, 2 * HD)], axis=0)  # [1536,1024]
        in_maps.append({
              "xt": np.ascontiguousarray(hidden_states[b].T).astype(
                  ml_dtypes.bfloat16),
              "wqkv": np.ascontiguousarray(wsel.T).astype(ml_dtypes.bfloat16),
              "owt": np.ascontiguousarray(o_w[:, rows].T).astype(
                  ml_dtypes.bfloat16),
              "cos": cos128,
              "sina": sina128,
        })

    trace = bool(int(os.environ.get("KERNEL_TRACE", "0")))
    try:
        last_results = run_bass_kernel_spmd(
            nc, in_maps, core_ids=list(range(N_CORES)), trace=trace)
    except ModuleNotFoundError:
        # axon NTFF hook unavailable in this container; run without trace
        last_results = run_bass_kernel_spmd(
            nc, in_maps, core_ids=list(range(N_CORES)), trace=False)

    out = np.empty((B, S, HD), dtype=np.float32)
    for b in range(B):
        out[b] = last_results.results[b * G]["out"]
        for g in range(1, G):
              out[b] += last_results.results[b * G + g]["out"]
    return out


# revision 33
# speedup vs baseline: 1.0385x; 1.0385x over previous
"""KindredAttention on 8 trn2 NeuronCores.

Sharding: core(b, g) = b*2 + g for batch b in 0..3, head-group g in 0..1
(8 heads per group). Data-parallel over batch, tensor-parallel over heads
(qkv column-split, o_proj row-split; host sums the two o_proj partials).

Per-core layouts (host pre-transposes so the kernel never transposes):
  xt   [1024, 2048] bf16 : hidden[b].T                  (d-major)
  wqkv [1024, 1536] bf16 : qkv_w rows for this group, transposed.
                           cols = [q(8 heads x 64) | k(...) | v(...)]
  owt  [512, 1024]  bf16 : o_w[:, group cols].T
  cos/sina [128, 2048] bf16 : RoPE tables (2 heads stacked, sign-folded sin)
  out  [2048, 1024] f32  : partial o_proj output (host adds g=0 + g=1)

v2 changes vs baseline:
  - Scores run head-PAIRS concurrently via PE row tiling (contraction=64:
    head 2f on array rows 0-63, head 2f+1 on rows 64-127, tile_position
    auto-derived from base partitions) -> scores at full PE rate.
  - One static PSUM layout for the whole kernel (pool A: 4x[128,512],
    pool B: 2x[128,1024] = 8 banks); phase-1 qkv, attention scores,
    attention PV and o_proj all share it, so no pool-close barriers.
  - wq / xt DMAs split per-dc (contiguous 384KB / 256KB chunks) so the
    first projection matmul issues after ~1 chunk instead of the full 5MB.
  - exp split per (chunk, head, j-half) across ACT(36)/DVE(28) per pair;
    softmax recip runs directly on PSUM row 64 (vector.reciprocal),
    denominator broadcast on GpSimd, normalize muls deferred into the
    next pair's stream so no engine FIFO stalls the PE.
  - phase-1 sb1 fc order interleaved q/k (0,4,1,5,...) so attention pair 0
    unblocks as soon as its own q/k rope is done.
"""

import os

import ml_dtypes
import numpy as np

H = 16
D = 64
BASE = 10000.0
B, S, HD = 4, 2048, 1024
G = 2          # head groups (tensor parallel)
HG = H // G    # heads per group = 8
N_CORES = 8

last_results = None  # stash for test.py (exec_time_ns etc.)


def _rope_tables():
    inv_freq = 1.0 / (BASE ** (np.arange(0, D, 2, dtype=np.float32) / D))
    t = np.arange(S, dtype=np.float32)
    freqs = np.outer(t, inv_freq)                       # [S, 32]
    emb = np.concatenate([freqs, freqs], -1)            # [S, 64]
    cos = np.cos(emb).T.astype(np.float32)              # [64, S]
    sin = np.sin(emb).T.astype(np.float32)
    sina = sin.copy()
    sina[:32] = -sina[:32]                              # rotate_half sign fold
    # swap 32-row halves: row p holds the factor applied to source row p, so
    # the DVE rot-mul reads src and table at the SAME base partition
    # (SBUF-SBUF TensorTensor requires equal input base partitions).
    sins = np.concatenate([sina[32:], sina[:32]], axis=0)
    cos128 = np.tile(cos, (2, 1)).astype(ml_dtypes.bfloat16).copy()
    sins128 = np.tile(sins, (2, 1)).astype(ml_dtypes.bfloat16).copy()
    return cos128, sins128


def _build():
    import concourse.mybir as mybir
    import concourse.tile as tile
    from concourse import bacc

    F32 = mybir.dt.float32
    BF16 = mybir.dt.bfloat16
    I16 = mybir.dt.int16
    Exp = mybir.ActivationFunctionType.Exp
    Copy = mybir.ActivationFunctionType.Copy
    Mult = mybir.AluOpType.mult
    Add = mybir.AluOpType.add

    # Schraudolph exp in bf16 bit space (scale 1/8 folded):
    #   bits16 = round(s * 16/ln2 + C)  ->  bitcast bf16 ~= exp(s/8)
    EXPA = 0.125 * 128.0 / float(np.log(2.0))
    EXPC = 16248.60

    nc = bacc.Bacc("TRN2", target_bir_lowering=False, debug=False,
                   num_devices=N_CORES)
    xt_d = nc.dram_tensor("xt", [HD, S], BF16, kind="ExternalInput")
    wq_d = nc.dram_tensor("wqkv", [HD, 3 * HG * D], BF16, kind="ExternalInput")
    ow_d = nc.dram_tensor("owt", [HG * D, HD], BF16, kind="ExternalInput")
    cos_d = nc.dram_tensor("cos", [128, S], BF16, kind="ExternalInput")
    sina_d = nc.dram_tensor("sina", [128, S], BF16, kind="ExternalInput")
    out_d = nc.dram_tensor("out", [S, HD], F32, kind="ExternalOutput")

    QB = S // 1024  # 2 big q/s blocks
    SC = S // 128   # 16 k-chunks
    DC = HD // 128  # 8 d-chunks

    with tile.TileContext(nc) as tc:
        with (
            tc.tile_pool(name="persist", bufs=1) as persist,
            tc.tile_pool(name="w1", bufs=1) as w1p,
            tc.tile_pool(name="xts", bufs=2) as xtp,
            tc.tile_pool(name="rope", bufs=5) as ropep,
            # PSUM: pool A = 4 x [128,512] (qkv proj j-halves, scores,
            # o_proj), pool B = 2 x [128,1024] (v proj, PV accum). 8 banks.
            tc.tile_pool(name="psA", bufs=1, space="PSUM") as psA,
            tc.tile_pool(name="psB", bufs=1, space="PSUM") as psB,
            tc.tile_pool(name="es", bufs=3) as esp,
            tc.tile_pool(name="nrm", bufs=1) as nrmp,
            tc.tile_pool(name="ot", bufs=2) as otp,
            tc.tile_pool(name="og", bufs=2) as ogp,
        ):
            # PE warm-up: ~20 throwaway matmuls on a zeroed tile while the
            # first DMAs land, so the HAM clock gate releases (K=8/8)
            # before real work starts instead of ~7us into it.
            wu_sb = ropep.tile([128, 512], BF16, tag="t00", name="wu_sb")
            nc.gpsimd.memset(wu_sb[:], 0.0)
            wu_ps = psA.tile([128, 2, 512], F32, tag="A0", name="wu_ps")
            for _ in range(20):
                nc.tensor.matmul(wu_ps[:, 0, :], wu_sb[:, 0:128], wu_sb[:],
                                 start=True, stop=True)

            # q rows: fc 0-3, k rows: fc 4-7 (feature-major, 2 heads/tile)
            qk_sb = persist.tile([128, 8, S], BF16, tag="qk")
            v_sb = persist.tile([128, SC, HG, D + 1], BF16, tag="v")
            nc.vector.memset(v_sb[:], 1.0)  # ones column survives at [..., 64]

            # RoPE tables land in compact [128,512] tiles per (sb, j):
            # reading 512-col slices of a 2048-wide tile runs the DVE
            # TensorTensor at ~1/3 rate (strided src), compact tiles don't.
            cos_t, sina_t = [], []
            for sj in range(4):
                c4 = w1p.tile([128, 512], BF16, tag=f"cos{sj}",
                              name=f"cos{sj}")
                s4_ = w1p.tile([128, 512], BF16, tag=f"sina{sj}",
                               name=f"sina{sj}")
                cos_t.append(c4)
                sina_t.append(s4_)
            ow_sb = w1p.tile([128, 4, HD], BF16, tag="ow")

            # ------------- DMA: per-dc chunks so compute starts early ------
            def load_xts(sb, wq_out=None):
                ssl = slice(sb * 1024, (sb + 1) * 1024)
                ts = []
                for dc in range(DC):
                    if wq_out is not None:  # interleave wq/xt per dc
                        wt = w1p.tile([128, 3 * HG * D], BF16,
                                      tag=f"wq{dc}", name=f"wq{dc}")
                        nc.sync.dma_start(
                            wt[:], wq_d[dc * 128:(dc + 1) * 128, :])
                        wq_out.append(wt)
                    xt_t = xtp.tile([128, 1024], BF16, tag=f"xt{dc}")
                    nc.sync.dma_start(
                        xt_t[:], xt_d[dc * 128:(dc + 1) * 128, ssl])
                    ts.append(xt_t)
                return ts

            wq_t = []
            xts0 = load_xts(0, wq_out=wq_t)
            for sj in range(4):
                ssl = slice(sj * 512, (sj + 1) * 512)
                nc.sync.dma_start(cos_t[sj][:], cos_d[:, ssl])
                nc.sync.dma_start(sina_t[sj][:], sina_d[:, ssl])
            nc.sync.dma_start(
                ow_sb[:], ow_d[:].rearrange("(a p) f -> p a f", p=128))



            # ---------------- phase 1: qkv projection + RoPE ----------------
            def fc_loop(sb, xts, fc_order):
                for fi, fc in enumerate(fc_order):
                    # 3-deep PSUM rotation: the B banks are idle until the
                    # v-section, so borrowing one gives the ACT drains two
                    # fc of slack instead of one.
                    if fi % 3 == 2:
                        psf = psB.tile([128, 1024], F32, tag="B0",
                                       name="psqk_b")
                        views = [psf[:, 0:512], psf[:, 512:1024]]
                    else:
                        psf = psA.tile([128, 2, 512], F32, tag=f"A{fi % 3}",
                                       name="psqk")
                        views = [psf[:, 0, :], psf[:, 1, :]]
                    for j in range(2):
                        for dc in range(DC):
                            nc.tensor.matmul(
                                views[j],
                                wq_t[dc][:, fc * 128:(fc + 1) * 128],
                                xts[dc][:, j * 512:(j + 1) * 512],
                                start=(dc == 0), stop=(dc == DC - 1),
                            )
                    for j in range(2):
                        sj = sb * 2 + j
                        jsl = slice(sb * 1024 + j * 512,
                                    sb * 1024 + (j + 1) * 512)
                        # drain to bf16 on ACT (idle in phase 1)
                        t0 = ropep.tile([128, 512], BF16, tag=f"t0{j}")
                        nc.scalar.activation(t0[:], views[j], Copy)
                        t1 = ropep.tile([128, 512], BF16, tag=f"t1{j}")
                        for r in (0, 64):  # two heads per chunk
                            nc.vector.tensor_mul(
                                t1[r:r + 32, :], t0[r + 32:r + 64, :],
                                sina_t[sj][r + 32:r + 64, :])
                            nc.vector.tensor_mul(
                                t1[r + 32:r + 64, :], t0[r:r + 32, :],
                                sina_t[sj][r:r + 32, :])
                        m1 = ropep.tile([128, 512], BF16, tag=f"m1{j}")
                        nc.vector.tensor_mul(m1[:], t0[:], cos_t[sj][:])
                        # final add on GpSimd: trims the DVE rope backlog so
                        # it drains before the attention exps need the DVE
                        nc.gpsimd.tensor_add(qk_sb[:, fc, jsl], m1[:], t1[:])

            def v_loop(sb, xts):
                for s4 in range(8):  # v s-chunks in this block
                    sc = sb * 8 + s4
                    ps = psB.tile([128, 1024], F32, tag=f"B{s4 % 2}")
                    for dc in range(DC):
                        nc.tensor.matmul(
                            ps[:, 0:512],
                            xts[dc][:, s4 * 128:(s4 + 1) * 128],
                            wq_t[dc][:, 1024:1536],
                            start=(dc == 0), stop=(dc == DC - 1),
                        )
                    nc.scalar.activation(
                        v_sb[:, sc, :, 0:D],
                        ps[:, 0:512].rearrange("p (h d) -> p h d", d=D),
                        Copy)

            xts1 = load_xts(1)
            fc_loop(0, xts0, [0, 4, 1, 5, 2, 6, 3, 7])
            v_loop(0, xts0)
            fc_loop(1, xts1, [0, 4, 1, 5, 2, 6, 3, 7])
            v_loop(1, xts1)  # 17us of DVE-free PE: drains the rope backlog

            # ---------------- phase 2+3: attention + o_proj ----------------
            # Per (q-block 512, head-pair f = heads 2f,2f+1):
            #   scores: K-chunk stationary [64,128] on array rows 0-63 (even
            #   head) / 64-127 (odd head) -> the two MMs run CONCURRENT via
            #   row tiling, into one [128, 2, 512] PSUM tile (pair side by
            #   side). ONE [128,1024] exp op per chunk, strictly alternating
            #   ACT (even c) / DVE (odd c) so the engines overlap, with a
            #   2-chunk PSUM pipeline. PV accumulates the ones-augmented V
            #   for both heads into one [128,1024] tile (head p at cols
            #   p*512) so the denominator row, broadcast and normalize run
            #   once per pair; normalize is an exact DVE divide (no recip).
            pending_norm = []  # deferred (ci_target, fn) normalize stages

            def flush_norm(ci=None):
                while pending_norm and (ci is None or pending_norm[0][0] <= ci):
                    pending_norm.pop(0)[1]()

            def att_unit(qb, qk2, f, ot, act_only=False):
                qsl = slice(qb * 1024 + qk2 * 512,
                            qb * 1024 + (qk2 + 1) * 512)
                osl = slice(qk2 * 512, (qk2 + 1) * 512)
                pv = psB.tile([128, 1024], F32, tag=f"B{f % 2}",
                              name="pv")
                es_t = [None] * SC
                for ci in range(SC + 2):
                    if ci < SC:
                        c = ci
                        csl = slice(c * 128, (c + 1) * 128)
                        # score PSUM rotation: depth 2 (A0/A1) for early
                        # chunks; from c>=8 the idle B bank (prev unit's
                        # pv, freed by its staged muls) joins -> depth 3,
                        # so exp(c) gates scores of c+3 instead of c+2.
                        # c=14/15 land on A0/A1 so the NEXT unit's first
                        # chunks don't wait on this unit's last exps.
                        tag3 = (f"A{c % 2}" if c < 8 else
                                ['A0', 'A1', 'B!'][(c - 8) % 3])
                        if tag3 == 'B!':
                            q_ps = psB.tile(
                                [128, 1024], F32,
                                tag=f"B{(f + 1) % 2}", name="qsb")
                            outs = [q_ps[:, 0:512], q_ps[:, 512:1024]]
                            flat = q_ps[:]
                        else:
                            q_ps = psA.tile([128, 2, 512], F32,
                                            tag=tag3, name="qs")
                            outs = [q_ps[:, 0, :], q_ps[:, 1, :]]
                            flat = q_ps[:].rearrange("p a b -> p (a b)")
                        for p in range(2):  # head parity: rows p*64
                            r = slice(p * 64, (p + 1) * 64)
                            nc.tensor.matmul(
                                outs[p],
                                qk_sb[r, 4 + f, csl],
                                qk_sb[r, f, qsl],
                                start=True, stop=True,
                            )
                        es2 = esp.tile([128, 2, 512], BF16,
                                       tag="es", name="es")
                        if c % 2 == 0 or act_only:  # ACT even / DVE odd
                            nc.scalar.activation(
                                es2[:].rearrange("p a b -> p (a b)"),
                                flat, Exp, scale=0.125)
                        else:
                            with nc.allow_low_precision(
                                    reason="schraudolph exp bf16"):
                                nc.vector.tensor_scalar(
                                    es2[:].rearrange(
                                        "p a b -> p (a b)"
                                    ).bitcast(I16),
                                    flat, EXPA, EXPC, Mult, Add)
                        es_t[c] = es2
                        flush_norm(ci)  # prev unit's staged chain
                    if ci >= 2:
                        c = ci - 2
                        for p in range(2):
                            nc.tensor.matmul(
                                pv[0:D + 1, p * 512:(p + 1) * 512],
                                v_sb[:, c, 2 * f + p, :],
                                es_t[c][:, p, :],
                                start=(c == 0), stop=(c == SC - 1),
                            )
                # softmax denominators (both heads at once): PSUM row
                # 64 -> SBUF on ACT, partition-broadcast on GpSimd,
                # reciprocal on the BROADCAST (full 64 partitions, much
                # cheaper than a 1-partition recip) on DVE, then two DVE
                # muls; the chain is staged into the next unit's stream
                # so no engine FIFO blocks.
                den = nrmp.tile([1, 1024], F32, tag="den")
                nc.scalar.activation(den[:], pv[D:D + 1, :], Copy)

                bc_box = []
                rc_box = []

                def st1(den_t=den, bc_box=bc_box):
                    bcs = nrmp.tile([64, 1024], F32, tag="bcs",
                                    name="bcs")
                    nc.gpsimd.partition_broadcast(bcs[:], den_t[:])
                    bc_box.append(bcs)

                def st2(bc_box=bc_box, rc_box=rc_box):
                    rcb = nrmp.tile([64, 1024], F32, tag="rcb",
                                    name="rcb")
                    with nc.allow_low_precision(reason="softmax recip"):
                        nc.vector.reciprocal_approx_fast(
                            rcb[:], bc_box[0][:])
                    rc_box.append(rcb)

                def st3(f=f, pv_t=pv, rc_box=rc_box, ot_t=ot, osl=osl):
                    for p in range(2):
                        ps512 = slice(p * 512, (p + 1) * 512)
                        nc.vector.tensor_mul(
                            ot_t[p * 64:(p + 1) * 64, f, osl],
                            pv_t[0:D, ps512], rc_box[0][:, ps512])
                pending_norm.extend([(1, st1), (4, st2), (6, st3)])

            def o_proj(qb, ot):
                # po shares pool-A banks with the unit scores
                for scq in range(8):
                    po = psA.tile([128, 2, 512], F32, tag=f"A{scq % 2}",
                                  name="po")
                    for jf in range(2):
                        jsl = slice(jf * 512, (jf + 1) * 512)
                        for oc in range(4):
                            nc.tensor.matmul(
                                po[:, jf, :],
                                ot[:, oc, scq * 128:(scq + 1) * 128],
                                ow_sb[:, oc, jsl],
                                start=(oc == 0), stop=(oc == 3),
                            )
                    og = ogp.tile([128, 1024], F32, tag="og")
                    nc.scalar.activation(
                        og[:], po[:].rearrange("p a b -> p (a b)"), Copy)
                    nc.sync.dma_start(
                        out_d[qb * 1024 + scq * 128:
                              qb * 1024 + (scq + 1) * 128, :], og[:])

            # Driver: qb0's o_proj is emitted AFTER qb1's first unit, so the
            # last qb0 unit's normalize chain (den->bcast->recip->muls,
            # ~6us of cross-engine latency) hides under that unit's PE work
            # instead of stalling the PE.  qb1's tail pays it once.
            ots = {0: otp.tile([128, 4, 1024], BF16, tag="ot", name="ot0"),
                   1: otp.tile([128, 4, 1024], BF16, tag="ot", name="ot1")}
            units = [(qb, qk2, f) for qb in range(QB) for qk2 in range(2)
                     for f in range(4)]
            for i, (qb, qk2, f) in enumerate(units):
                # the very first unit overlaps sb1's DVE rope tail: keep its
                # exps off the DVE so nothing stalls
                att_unit(qb, qk2, f, ots[qb],
                         act_only=(i == 0))
                if i == 9:
                    o_proj(0, ots[0])
            flush_norm()
            o_proj(1, ots[1])

    nc.compile()
    return nc


def kernel(hidden_states, qkv_w, o_w):
    global last_results
    from concourse.bass_utils import run_bass_kernel_spmd

    hidden_states = np.asarray(hidden_states, dtype=np.float32)
    qkv_w = np.asarray(qkv_w, dtype=np.float32)
    o_w = np.asarray(o_w, dtype=np.float32)

    cos128, sina128 = _rope_tables()
    nc = _build()

    in_maps = []
    for core in range(N_CORES):
        b, g = core // G, core % G
        heads = range(g * HG, (g + 1) * HG)
        rows = np.concatenate(
              [np.arange(h * D, (h + 1) * D) for h in heads])
        wsel = np.concatenate(
              [qkv_w[off + rows] for off in (0, HD, 2 * HD)], axis=0)  # [1536,1024]
        in_maps.append({
              "xt": np.ascontiguousarray(hidden_states[b].T).astype(
                  ml_dtypes.bfloat16),
              "wqkv": np.ascontiguousarray(wsel.T).astype(ml_dtypes.bfloat16),
              "owt": np.ascontiguousarray(o_w[:, rows].T).astype(
                  ml_dtypes.bfloat16),
              "cos": cos128,
              "sina": sina128,
        })

    trace = bool(int(os.environ.get("KERNEL_TRACE", "0")))
    try:
        last_results = run_bass_kernel_spmd(
            nc, in_maps, core_ids=list(range(N_CORES)), trace=trace)
    except ModuleNotFoundError:
        # axon NTFF hook unavailable in this container; run without trace
        last_results = run_bass_kernel_spmd(
            nc, in_maps, core_ids=list(range(N_CORES)), trace=False)

    out = np.empty((B, S, HD), dtype=np.float32)
    for b in range(B):
        out[b] = last_results.results[b * G]["out"]
        for g in range(1, G):
              out[b] += last_results.results[b * G + g]["out"]
    return out
